# revision 62
# baseline (speedup 1.0000x reference)
"""Multi-head attention (B=512,S=64,D=1024,H=16) on 8 trn2 NeuronCores.

Strategy: pure data-parallel over the batch dim — each core gets 64 batches
(4096 tokens) and runs the full fused MHA layer locally; no collectives.

Per-core dataflow (token chunks of 512 = 8 batches):
  x [tok,1024] --PE transpose--> xT [1024,tok] (feature-major, bf16)
  xT --Pool cast--> x8 (fp8e4, DoubleRow [128,2,512] layout)
  qT = (64*Wq).T @ x8, kT = (64*Wk).T @ x8   fp8 DoubleRow matmuls (~1.5x bf16)
  v  = x @ Wv bf16                            (token-major, ones col interleaved)
  scoresT[k,(h,q)] = kT.T @ qT per (batch,head) -> ONE [128,128] psum bank
  es = exp(scoresT * scale/4096)              single ACT exp per (u,t)
  ctx[q,:]|sumexp[q] = es.T @ [v|1]           -> recip + per-partition scale
  ctxT via PE transpose (4-packed psum); out = ctx @ Wo staged to otmp (SBUF),
  gelu batched per block (minimizes ACT table swaps), DMA out.

Numerics: fp8e4m3 only on the Q/K projections (softmax damps the error;
simulated rel-err 6.7e-3 vs 2e-2 budget). V/O projections + scores/ctx stay
bf16 with fp32 PSUM accumulation.

PSUM packing rule (hardware): concurrent matmuls may share a PSUM bank only
if they use the same array row-strip (same operand base partition) or a
strict diagonal (row,col) placement. The [128,128] score bank is written
hh-outer so adjacent matmuls are same-row-strip or diagonal.
"""

import sys

sys.path.insert(0, "/opt/trn_rl_repo")

import numpy as np

import concourse.bass as bass
import concourse.tile as tile
from concourse import mybir
from concourse.bass_utils import run_bass_kernel_spmd
from concourse.masks import make_identity

F32 = mybir.dt.float32
BF = mybir.dt.bfloat16
F8 = mybir.dt.float8e4

B, S, D, H = 512, 64, 1024, 16
DH = D // H  # 64
NCORES = 8
BL = B // NCORES  # 64 batches per core
NTOK = BL * S  # 4096 tokens per core
CHUNK = 512  # tokens per pipeline chunk (8 batches)
NCH = NTOK // CHUNK  # 8
TT = CHUNK // 128  # 4 token-tiles per chunk
KT = D // 128  # 8 d-tiles
NG = KT // 2  # 4 DoubleRow k-groups
SCALE = 1.0 / np.sqrt(np.float32(D))  # 1/32
WS = 64.0  # fp8 weight pre-scale (keeps w out of e4m3 subnormals)


def _split_multiwait(nc, limit=1):
    """walrus can emit at most one sync-wait per instruction; TileContext's
    tail drain carries one wait per touched processor. Hoist extras onto
    chained NOPs."""
    f = nc.m.functions[0]
    for blk in f.blocks:
        new_insts = []
        for inst in blk.instructions:
            si = inst.sync_info
            if si is not None and len(si.on_wait) > limit:
                extra = si.on_wait[:-limit]
                keep = si.on_wait[-limit:]
                for i, w in enumerate(extra):
                    nop = mybir.InstNoOp(
                        name=f"{inst.name}-waitsplit{i}",
                        sync_info=mybir.SyncInfo(on_wait=[w], on_update=[]),
                        bass_nofuse=True,
                        ins=[],
                        outs=[],
                    )
                    nop.engine = inst.engine
                    new_insts.append(nop)
                si.on_wait[:] = keep
            new_insts.append(inst)
        blk.instructions[:] = new_insts


def _interleave(a, b):
    """Merge two unit lists round-robin, proportionally to their lengths."""
    out = []
    ia = ib = 0
    la, lb = len(a), len(b)
    while ia < la or ib < lb:
        if ib >= lb or (ia < la and ia * lb <= ib * la):
            out.append(a[ia])
            ia += 1
        else:
            out.append(b[ib])
            ib += 1
    return out


def build(
    fp8_qk=True,
    split_waits=True,
    pack_transpose=True,
    paired_scores=True,
    pool_casts=True,
):
    nc = bass.Bass("TRN2", debug=False, num_devices=NCORES)

    x_d = nc.declare_dram_parameter("x", [NTOK, D], F32, isOutput=False)
    w_d = {}
    b_d = {}
    for nm in ("wq", "wk", "wv", "wo"):
        w_d[nm] = nc.declare_dram_parameter(f"{nm}_w", [D, D], F32, isOutput=False)
        b_d[nm] = nc.declare_dram_parameter(f"{nm}_b", [D], F32, isOutput=False)
    out_d = nc.declare_dram_parameter("out", [NTOK, D], F32, isOutput=True)

    exp_scale = float(SCALE / (WS * WS)) if fp8_qk else float(SCALE)

    with tile.TileContext(nc) as tc:
        with (
            tc.tile_pool(name="weights", bufs=1) as wpool,
            tc.tile_pool(name="consts", bufs=1) as cpool,
            tc.tile_pool(name="wload", bufs=2) as ldpool,
            tc.tile_pool(name="xin", bufs=2) as xpool,
            tc.tile_pool(name="feat", bufs=2) as fpool,
            tc.tile_pool(name="attn", bufs=4) as apool,
            tc.tile_pool(name="outb", bufs=2) as opool,
            tc.tile_pool(name="psum", bufs=2, space="PSUM") as ppool,
        ):
            wt = {nm: [None] * KT for nm in ("wq", "wk", "wv", "wo")}  # bf16 strips
            w8 = {nm: [None] * NG for nm in ("wq", "wk")}  # fp8 DR groups
            biases = {}
            consts = {}

            def unit_load_w8(nm, g):
                """fp8 DoubleRow weight group g: rows [256g, 256g+256) as
                [128 part, 2 ktile, 1024] scaled by WS."""

                def f():
                    wb = wpool.tile([128, 2 * D], F8, tag=f"w8_{nm}_{g}", name=f"w8{nm}{g}")
                    for a in range(2):
                        wf = ldpool.tile([128, D], F32, tag="wload", bufs=4, name="wf8")
                        nc.sync.dma_start(
                            out=wf[:],
                            in_=w_d[nm][
                                g * 256 + a * 128 : g * 256 + (a + 1) * 128, :
                            ],
                        )
                        # DVE/ACT, not Pool: the Q7 fp8-convert path is ~15x slower
                        if a == 0:
                            nc.vector.tensor_scalar(
                                out=wb[:, a * D : (a + 1) * D],
                                in0=wf[:],
                                scalar1=WS,
                                scalar2=None,
                                op0=mybir.AluOpType.mult,
                            )
                        else:
                            nc.scalar.activation(
                                out=wb[:, a * D : (a + 1) * D],
                                in_=wf[:],
                                func=mybir.ActivationFunctionType.Copy,
                                scale=WS,
                            )
                    w8[nm][g] = wb

                return f

            def unit_load_weight(nm, k):
                def f():
                    wf = ldpool.tile([128, D], F32, tag="wload", bufs=4, name="wf")
                    nc.sync.dma_start(
                        out=wf[:], in_=w_d[nm][k * 128 : (k + 1) * 128, :]
                    )
                    wb = wpool.tile([128, D], BF, tag=f"w_{nm}_{k}", name=f"w{nm}{k}")
                    if k % 2 == 0:
                        nc.vector.tensor_copy(out=wb[:], in_=wf[:])
                    else:
                        nc.scalar.activation(
                            out=wb[:], in_=wf[:], func=mybir.ActivationFunctionType.Copy
                        )
                    wt[nm][k] = wb

                return f

            def unit_biases():
                def f():
                    # per-partition (feature-major) bias layout for q/k evac;
                    # pre-scaled by WS to match the scaled fp8 projections
                    for nm in ("wq", "wk"):
                        braw = ldpool.tile([128, KT], F32, tag="brawqk", name="braw")
                        nc.sync.dma_start(
                            out=braw[:], in_=b_d[nm][:].rearrange("(m p) -> p m", p=128)
                        )
                        bt = cpool.tile([128, KT], F32, tag=f"{nm}_pb", name=f"{nm}_pb")
                        nc.vector.tensor_scalar(
                            out=bt[:],
                            in0=braw[:],
                            scalar1=WS if fp8_qk else 1.0,
                            scalar2=None,
                            op0=mybir.AluOpType.mult,
                        )
                        biases[nm] = bt
                    # broadcast-to-all-partitions bias tiles for v/o
                    ones_col = cpool.tile([1, 128], BF, tag="ones_col", name="ones_col")
                    nc.gpsimd.memset(ones_col[:], 1.0)
                    for nm in ("wv", "wo"):
                        row = ldpool.tile([1, D], F32, tag="wload", bufs=4, name="row")
                        nc.sync.dma_start(out=row[:], in_=b_d[nm][:].unsqueeze(0))
                        row_bf = ldpool.tile([1, D], BF, tag="rowbf", name="row_bf")
                        nc.vector.tensor_copy(out=row_bf[:], in_=row[:])
                        bc = cpool.tile([128, D], F32, tag=f"{nm}_bc", name=f"{nm}_bc")
                        for n in range(2):
                            psb = ppool.tile([128, 512], F32, tag="proj", bufs=2, name="psb")
                            nc.tensor.matmul(
                                psb[:],
                                lhsT=ones_col[:],
                                rhs=row_bf[:, n * 512 : (n + 1) * 512],
                                start=True,
                                stop=True,
                            )
                            nc.vector.tensor_copy(
                                out=bc[:, n * 512 : (n + 1) * 512], in_=psb[:]
                            )
                        biases[nm] = bc

                return f

            live = {}  # per-chunk tiles handed between stages

            # ---------------- stage A: x load / transpose / fp8 cast ------
            def u_x(ch, t):
                """DMA one 128-token tile + cast to bf16 (alternating DVE/Pool)."""
                tok0 = ch * CHUNK

                def f():
                    st = live.setdefault(ch, {})
                    if "xb" not in st:
                        st["xb"] = [None] * TT
                    xf = xpool.tile([128, D], F32, tag="xf32", name="xf")
                    nc.sync.dma_start(
                        out=xf[:], in_=x_d[tok0 + t * 128 : tok0 + (t + 1) * 128, :]
                    )
                    xb = xpool.tile([128, D], BF, tag=f"xbf{t}", bufs=1, name=f"xb{t}")
                    eng = nc.gpsimd if (pool_casts and t % 2 == 1) else nc.vector
                    eng.tensor_copy(out=xb[:], in_=xf[:])
                    st["xb"][t] = xb

                return f

            def u_xT(ch, k):
                """4 PE transposes of feature-block k (all 4 token tiles)
                through one [128,512] psum tile; single evac copy."""

                def f():
                    st = live[ch]
                    if "xT" not in st:
                        st["xT"] = [
                            fpool.tile([128, CHUNK], BF, tag=f"xT{i}", name=f"xT{i}")
                            for i in range(KT)
                        ]
                    if fp8_qk and "x8" not in st:
                        st["x8"] = [
                            fpool.tile(
                                [128, 2 * CHUNK], F8, tag=f"x8{i}", bufs=1, name=f"x8{i}"
                            )
                            for i in range(NG)
                        ]
                    if pack_transpose:
                        ps = ppool.tile([128, CHUNK], BF, tag="tp", bufs=2, name="ps_tp")
                        for t in range(TT):
                            nc.tensor.transpose(
                                ps[:, t * 128 : (t + 1) * 128],
                                st["xb"][t][:, k * 128 : (k + 1) * 128],
                                consts["identity"],
                            )
                        nc.vector.tensor_copy(out=st["xT"][k][:], in_=ps[:])
                        if fp8_qk:
                            # second evac of the same psum, straight to fp8
                            x8_dst = st["x8"][k // 2][
                                :, (k % 2) * CHUNK : (k % 2 + 1) * CHUNK
                            ]
                            if k % 2 == 0:
                                nc.scalar.activation(
                                    out=x8_dst,
                                    in_=ps[:],
                                    func=mybir.ActivationFunctionType.Copy,
                                )
                            else:
                                nc.vector.tensor_copy(out=x8_dst, in_=ps[:])
                    else:
                        for t in range(TT):
                            ps = ppool.tile([128, 128], BF, tag="tp", bufs=2, name="ps_tp")
                            nc.tensor.transpose(
                                ps[:],
                                st["xb"][t][:, k * 128 : (k + 1) * 128],
                                consts["identity"],
                            )
                            nc.vector.tensor_copy(
                                out=st["xT"][k][:, t * 128 : (t + 1) * 128], in_=ps[:]
                            )
                        if fp8_qk:
                            nc.vector.tensor_copy(
                                out=st["x8"][k // 2][
                                    :, (k % 2) * CHUNK : (k % 2 + 1) * CHUNK
                                ],
                                in_=st["xT"][k][:],
                            )

                return f

            # ---------------- stage B: dense projections ------------------
            def u_qk(ch, which, m):
                def f():
                    st = live[ch]
                    key = "qT" if which == "wq" else "kT"
                    if key not in st:
                        st[key] = [
                            fpool.tile([128, CHUNK], BF, tag=f"{key}{i}", name=f"{key}{i}")
                            for i in range(KT)
                        ]
                    ps = ppool.tile([128, CHUNK], F32, tag="proj", bufs=2, name="ps_qk")
                    if fp8_qk:
                        for g in range(NG):
                            nc.tensor.matmul(
                                ps[:],
                                lhsT=w8[which][g][:]
                                .rearrange("p (a d) -> p a d", a=2)[
                                    :, :, m * 128 : (m + 1) * 128
                                ],
                                rhs=st["x8"][g][:].rearrange("p (a n) -> p a n", a=2),
                                start=(g == 0),
                                stop=(g == NG - 1),
                                perf_mode=mybir.MatmulPerfMode.DoubleRow,
                            )
                    else:
                        for k in range(KT):
                            nc.tensor.matmul(
                                ps[:],
                                lhsT=wt[which][k][:, m * 128 : (m + 1) * 128],
                                rhs=st["xT"][k][:],
                                start=(k == 0),
                                stop=(k == KT - 1),
                            )
                    # all on DVE: keeps the ACT queue clear for the exps that
                    # gate the attention stream
                    nc.vector.tensor_scalar(
                        out=st[key][m][:],
                        in0=ps[:],
                        scalar1=biases[which][:, m : m + 1],
                        scalar2=None,
                        op0=mybir.AluOpType.add,
                    )

                return f

            def u_v(ch, t, n):
                def f():
                    st = live[ch]
                    if "vaug" not in st:
                        st["vaug"] = [
                            apool.tile(
                                [128, H * (DH + 1)], BF,
                                tag=f"vaug{i}", name=f"vaug{i}", bufs=2,
                            )
                            for i in range(TT)
                        ]
                        for i in range(TT):
                            nc.gpsimd.memset(
                                st["vaug"][i][:]
                                .rearrange("p (h c) -> p h c", c=DH + 1)[:, :, DH : DH + 1],
                                1.0,
                            )
                    ps = ppool.tile([128, CHUNK], F32, tag="proj", bufs=2, name="ps_v")
                    for k in range(KT):
                        nc.tensor.matmul(
                            ps[:],
                            lhsT=st["xT"][k][:, t * 128 : (t + 1) * 128],
                            rhs=wt["wv"][k][:, n * 512 : (n + 1) * 512],
                            start=(k == 0),
                            stop=(k == KT - 1),
                        )
                    nc.vector.tensor_tensor(
                        out=st["vaug"][t][:]
                        .rearrange("p (h c) -> p h c", c=DH + 1)[:, n * 8 : (n + 1) * 8, 0:DH],
                        in0=ps[:].rearrange("p (j c) -> p j c", c=DH),
                        in1=biases["wv"][:, n * 512 : (n + 1) * 512].rearrange(
                            "p (j c) -> p j c", c=DH
                        ),
                        op=mybir.AluOpType.add,
                    )

                return f

            # ---------------- stage C: attention --------------------------
            es_tiles = {}

            def u_scores2(ch, u, tp, hh):
                """Scores for head-pair slot hh of BOTH t=2tp and t=2tp+1 with
                M=N=128 (both batch halves): the cross-batch quadrants are
                garbage but ctx only ever reads the same-batch (row, col)
                blocks. 2 same-row-strip matmuls into one [128,256] psum bank;
                single batched exp."""

                def f():
                    st = live[ch]
                    qT, kT = st["qT"], st["kT"]
                    es = apool.tile([128, 128], BF, tag="expS", bufs=6, name="es")
                    es_tiles[(ch, u, tp, hh)] = es
                    hsl = slice(hh * 64, hh * 64 + 64)
                    ps_s = ppool.tile([128, 128], F32, tag="sc", bufs=2, name="ps_s")
                    for j in (0, 1):
                        t = 2 * tp + j
                        for bpar in (0, 1):
                            toksl = slice(u * 128 + bpar * 64, u * 128 + bpar * 64 + 64)
                            nc.tensor.matmul(
                                ps_s[bpar * 64 : bpar * 64 + 64, j * 64 : j * 64 + 64],
                                lhsT=kT[t][hsl, toksl],
                                rhs=qT[t][hsl, toksl],
                                start=True,
                                stop=True,
                            )
                    nc.scalar.activation(
                        out=es[:],
                        in_=ps_s[:],
                        func=mybir.ActivationFunctionType.Exp,
                        scale=exp_scale,
                    )

                return f

            def u_scores(ch, u, t):
                """Unpaired fallback: per (u,t), one [128,64] bank per hh."""

                def f():
                    st = live[ch]
                    qT, kT = st["qT"], st["kT"]
                    es = apool.tile([128, 128], BF, tag="expS", name="es")
                    es_tiles[(ch, u, t)] = es
                    for hh in (0, 1):
                        hsl = slice(hh * 64, hh * 64 + 64)
                        ps_s = ppool.tile([128, 64], F32, tag="sc", bufs=2, name="ps_s")
                        for bpar in (0, 1):
                            toksl = slice(u * 128 + bpar * 64, u * 128 + bpar * 64 + 64)
                            nc.tensor.matmul(
                                ps_s[bpar * 64 : bpar * 64 + 64, :],
                                lhsT=kT[t][hsl, toksl],
                                rhs=qT[t][hsl, toksl],
                                start=True,
                                stop=True,
                            )
                        nc.scalar.activation(
                            out=es[:, hsl],
                            in_=ps_s[:],
                            func=mybir.ActivationFunctionType.Exp,
                            scale=exp_scale,
                        )

                return f

            def u_ctx(ch, u, t):
                def f():
                    st = live[ch]
                    if "ctx" not in st:
                        st["ctx"] = [
                            apool.tile(
                                [128, D], BF, tag=f"ctx{i}", name=f"ctx{i}",
                                bufs=2 if fp8_qk else 1,
                            )
                            for i in range(TT)
                        ]
                    vaug, ctx = st["vaug"], st["ctx"]
                    es_t = {}
                    es_off = {}
                    if paired_scores:
                        tp, j = t // 2, t % 2
                        for hh in (0, 1):
                            key = (ch, u, tp, hh)
                            es_t[hh] = es_tiles.pop(key) if j == 1 else es_tiles[key]
                            es_off[hh] = j * 64
                    else:
                        es = es_tiles.pop((ch, u, t))
                        for hh in (0, 1):
                            es_t[hh] = es
                            es_off[hh] = hh * 64
                    ps_c = ppool.tile([128, 130], F32, tag="cx", bufs=2, name="ps_c")
                    for bpar in (0, 1):
                        bsl = slice(bpar * 64, bpar * 64 + 64)
                        for hh in (0, 1):
                            h = 2 * t + hh
                            off = es_off[hh]
                            nc.tensor.matmul(
                                ps_c[bsl, hh * 65 : hh * 65 + 65],
                                lhsT=es_t[hh][bsl, off : off + 64],
                                rhs=vaug[u][bsl, h * 65 : (h + 1) * 65],
                                start=True,
                                stop=True,
                            )
                    rc = apool.tile([128, 2], F32, tag="recip", name="rc")
                    nc.vector.reciprocal(rc[:], ps_c[:, DH : 2 * 65 : 65])
                    # normalize: hh0 on DVE, hh1 on ACT (Identity w/ scale AP)
                    nc.vector.tensor_scalar(
                        out=ctx[u][:, 2 * t * DH : (2 * t + 1) * DH],
                        in0=ps_c[:, 0:DH],
                        scalar1=rc[:, 0:1],
                        scalar2=None,
                        op0=mybir.AluOpType.mult,
                    )
                    nc.scalar.activation(
                        out=ctx[u][:, (2 * t + 1) * DH : (2 * t + 2) * DH],
                        in_=ps_c[:, 65 : 65 + DH],
                        func=mybir.ActivationFunctionType.Identity,
                        scale=rc[:, 1:2],
                    )

                return f

            def u_cT(ch, p, k):
                """Transpose ctx token-tiles 2p,2p+1, feature block k, through
                a [128,256] slice of the tp psum; single evac copy."""

                def f():
                    st = live[ch]
                    if "cT" not in st:
                        st["cT"] = [
                            fpool.tile([128, CHUNK], BF, tag=f"cT{i}", bufs=1, name=f"cT{i}")
                            for i in range(KT)
                        ]
                    if pack_transpose:
                        ps = ppool.tile([128, CHUNK], BF, tag="tp", bufs=2, name="ps_tpc")
                        for j in range(2):
                            u = 2 * p + j
                            nc.tensor.transpose(
                                ps[:, j * 128 : (j + 1) * 128],
                                st["ctx"][u][:, k * 128 : (k + 1) * 128],
                                consts["identity"],
                            )
                        nc.vector.tensor_copy(
                            out=st["cT"][k][:, p * 256 : (p + 1) * 256], in_=ps[:, 0:256]
                        )
                    else:
                        for j in range(2):
                            u = 2 * p + j
                            ps = ppool.tile([128, 128], BF, tag="tp", bufs=2, name="ps_tpc")
                            nc.tensor.transpose(
                                ps[:],
                                st["ctx"][u][:, k * 128 : (k + 1) * 128],
                                consts["identity"],
                            )
                            nc.vector.tensor_copy(
                                out=st["cT"][k][:, u * 128 : (u + 1) * 128], in_=ps[:]
                            )

                return f

            # ---------------- stage D: output projection + gelu -----------
            def u_out(ch, t, n):
                def f():
                    st = live[ch]
                    if "otmp" not in st:
                        st["otmp"] = {}
                    ps = ppool.tile([128, CHUNK], F32, tag="proj", bufs=2, name="ps_o")
                    for k in range(KT):
                        nc.tensor.matmul(
                            ps[:],
                            lhsT=st["cT"][k][:, t * 128 : (t + 1) * 128],
                            rhs=wt["wo"][k][:, n * 512 : (n + 1) * 512],
                            start=(k == 0),
                            stop=(k == KT - 1),
                        )
                    tmp = opool.tile([128, 512], F32, tag=f"otmp{t}{n}", bufs=1, name="tmp")
                    nc.vector.tensor_tensor(
                        out=tmp[:],
                        in0=ps[:],
                        in1=biases["wo"][:, n * 512 : (n + 1) * 512],
                        op=mybir.AluOpType.add,
                    )
                    st["otmp"][(t, n)] = tmp

                return f

            def u_gelu(ch, t, n):
                tok0 = ch * CHUNK

                def f():
                    st = live[ch]
                    tmp = st["otmp"].pop((t, n))
                    og = opool.tile([128, 512], F32, tag="ogelu", bufs=2, name="og")
                    nc.scalar.activation(
                        out=og[:], in_=tmp[:], func=mybir.ActivationFunctionType.Gelu
                    )
                    nc.sync.dma_start(
                        out=out_d[
                            tok0 + t * 128 : tok0 + (t + 1) * 128,
                            n * 512 : (n + 1) * 512,
                        ],
                        in_=og[:],
                    )

                return f

            # ---------------- unit-list builders --------------------------
            def attn_full(ch):
                """Full attention + output stream for chunk ch: scores
                staggered 2 ahead of ctx; cT(p0) woven into u2/u3; out(0..1)
                strictly after cT(p0), out(2..3) strictly after cT(p1);
                gelus after their out units."""
                per_u = []
                for u in range(TT):
                    if paired_scores:
                        # s2(tp,hh) covers t=2tp,2tp+1 for one partition half;
                        # ctx(t) trails the pair covering it by >=2 units so
                        # the ACT exp latency is fully hidden
                        seq = [
                            u_scores2(ch, u, 0, 0),
                            u_scores2(ch, u, 0, 1),
                            u_scores2(ch, u, 1, 0),
                            u_scores2(ch, u, 1, 1),
                        ]
                        for t in range(KT - 4):
                            seq.append(u_ctx(ch, u, t))
                            tpn = (t + 4) // 2
                            hhn = (t + 4) % 2
                            seq.append(u_scores2(ch, u, tpn, hhn))
                        for t in range(KT - 4, KT):
                            seq.append(u_ctx(ch, u, t))
                    else:
                        seq = [u_scores(ch, u, 0), u_scores(ch, u, 1)]
                        for t in range(KT - 2):
                            seq.append(u_scores(ch, u, t + 2))
                            seq.append(u_ctx(ch, u, t))
                        seq.append(u_ctx(ch, u, KT - 2))
                        seq.append(u_ctx(ch, u, KT - 1))
                    per_u.append(seq)
                tail0 = [u_cT(ch, 0, k) for k in range(KT)]
                tail1 = [u_cT(ch, 1, k) for k in range(KT)]
                out01 = [u_out(ch, t, n) for t in (0, 1) for n in range(2)]
                out23 = [u_out(ch, t, n) for t in (2, 3) for n in range(2)]
                units = per_u[0] + per_u[1]
                units += _interleave(per_u[2] + per_u[3], tail0)
                units += _interleave(tail1, out01)
                units += out23
                return units

            def heavy_list(ch, with_xT_next):
                # v + xT first: they only need bf16 xT, giving the engine
                # queues time to finish x8(ch) casts before the qk strips
                # need them; qk last also finishes right before attn(ch)
                units = []
                for m in range(KT):
                    units.append(u_qk(ch, "wq", m))
                    units.append(u_qk(ch, "wk", m))
                mids = []
                if with_xT_next:
                    mids.extend(u_xT(ch + 1, k) for k in range(KT))
                vs = [u_v(ch, t, n) for t in range(TT) for n in range(2)]
                units.extend(_interleave(vs, mids))
                return units

            def light_list(ch, with_x_next, with_attn_prev):
                # gelus of chunk ch-2 lead the list: their otmp inputs are
                # long ready, so they never head-of-line block the ACT queue
                units = []
                if with_x_next:
                    units.extend(u_x(ch + 1, t) for t in range(TT))
                if ch >= 2:
                    units.extend(u_gelu(ch - 2, t, n) for t in range(TT) for n in range(2))
                if with_attn_prev:
                    units.extend(attn_full(ch - 1))
                return units

            # ---- emission ----
            identity = cpool.tile([128, 128], BF, tag="ident", name="identity")
            make_identity(nc, identity[:])
            consts["identity"] = identity

            # prologue: x(0) + wq8 first (Q(0) can start as soon as they land)
            for t in range(TT):
                u_x(0, t)()
            if fp8_qk:
                for g in range(NG):
                    unit_load_w8("wq", g)()
            else:
                for k in range(KT):
                    unit_load_weight("wq", k)()
            unit_biases()()
            for k in range(KT):
                u_xT(0, k)()
            if fp8_qk:
                wk_units = [unit_load_w8("wk", g) for g in range(NG)]
            else:
                wk_units = [unit_load_weight("wk", k) for k in range(KT)]
            q0 = [u_qk(0, "wq", m) for m in range(KT)]
            k0 = [u_qk(0, "wk", m) for m in range(KT)]
            for u in _interleave(q0, wk_units):
                u()
            for u in _interleave(
                k0, [unit_load_weight("wv", k) for k in range(KT)]
            ):
                u()
            # v(0) then xT(1) heavy units; x(1) loads first in the light list
            h0 = [u_v(0, t, n) for t in range(TT) for n in range(2)] + [
                u_xT(1, k) for k in range(KT)
            ]
            l0 = [u_x(1, t) for t in range(TT)] + [
                unit_load_weight("wo", k) for k in range(KT)
            ]
            for u in _interleave(h0, l0):
                u()

            # steady state
            for ch in range(1, NCH):
                hl = heavy_list(ch, with_xT_next=(ch + 1 < NCH))
                ll = light_list(ch, with_x_next=(ch + 1 < NCH), with_attn_prev=True)
                for u in _interleave(hl, ll):
                    u()
                if ch >= 2:
                    live.pop(ch - 2)

            # epilogue: last chunk's attention + output; final gelus woven
            # into the last out units to shorten the ACT-serial tail
            for u in [u_gelu(NCH - 2, t, n) for t in range(TT) for n in range(2)]:
                u()
            al = attn_full(NCH - 1)
            for u in al[:-4]:
                u()
            for u in _interleave(
                al[-4:], [u_gelu(NCH - 1, t, n) for t in range(TT) for n in range(2)]
            ):
                u()
            live.pop(NCH - 2)
            live.pop(NCH - 1)

    if split_waits:
        _split_multiwait(nc)
    return nc


_NC = {}


def _get_nc(fp8_qk=True):
    if fp8_qk not in _NC:
        _NC[fp8_qk] = build(fp8_qk=fp8_qk)
    return _NC[fp8_qk]


def _make_in_maps(inputs):
    x = np.ascontiguousarray(np.asarray(inputs["x"], dtype=np.float32))
    full = {
        nm: np.ascontiguousarray(np.asarray(inputs[nm], dtype=np.float32))
        for nm in ("wq_w", "wq_b", "wk_w", "wk_b", "wv_w", "wv_b", "wo_w", "wo_b")
    }
    in_maps = []
    for c in range(NCORES):
        m = {"x": np.ascontiguousarray(x[c * BL : (c + 1) * BL].reshape(NTOK, D))}
        m.update(full)
        in_maps.append(m)
    return in_maps


def kernel(**inputs):
    nc = _get_nc()
    res = run_bass_kernel_spmd(
        nc, _make_in_maps(inputs), core_ids=list(range(NCORES))
    ).results
    parts = [res[c]["out"].reshape(BL, 8, 8, D) for c in range(NCORES)]
    return np.concatenate(parts, axis=0)


def kernel_profiled(**inputs):
    """Like kernel() but requests an NTFF trace; returns (out, exec_time_ns, raw)."""
    nc = _get_nc()
    r = run_bass_kernel_spmd(
        nc, _make_in_maps(inputs), core_ids=list(range(NCORES)), trace=True
    )
    parts = [r.results[c]["out"].reshape(BL, 8, 8, D) for c in range(NCORES)]
    return np.concatenate(parts, axis=0), r.exec_time_ns, r
